# revision 1
# baseline (speedup 1.0000x reference)
"""LoRA gather-BMM + dense GEMM kernel for Trainium2 (8 NeuronCores).

Computation (per the module semantics):
    A = lora_A[wids]; Bw = lora_B[wids]
    y = (x @ A) @ Bw * 2 + x @ M          # x: [B, 1, IN]

Distribution: data-parallel over batch. Each of the 8 cores processes
B/8 = 256 samples and reads the full (small) adapter banks and M.
No collectives; per-core outputs are concatenated on the host.

Per-core algorithm (all PE work in fp16 with fp32 PSUM accumulation):
  1. H^T = A_all^T @ x^T   for ALL 64 adapters  -> [1024, 256] (rank-major)
  2. h^T = H^T * mask      where mask[p, j] = (wids[j] == row_adapter[p]);
     this realizes the gather densely (row_adapter is an iota constant).
  3. y   = x @ M + h_exp @ (2 * B_all)   accumulated in PSUM, drained to fp16.
"""

import numpy as np

import concourse.bacc as bacc
import concourse.mybir as mybir
import concourse.tile as tile
from concourse.bass_utils import run_bass_kernel_spmd

B, IN, R, OUT, NA = 2048, 4096, 16, 4096, 64
N_CORES = 8
BC = B // N_CORES          # 256 samples per core
P = 128
KT = IN // P               # 32 contraction tiles over IN
NR = NA * R                # 1024 stacked rank rows
RT = NR // P               # 8 contraction tiles over rank
NH = 2                     # halves of OUT per PSUM pass
HW = OUT // NH             # 2048
NS = HW // 512             # 4 free-dim slices of 512
MB = BC // P               # 2 batch tiles

F16 = mybir.dt.float16
F32 = mybir.dt.float32


def build_nc(loop_n=None, staggered=False):
    nc = bacc.Bacc(
        "TRN2",
        target_bir_lowering=False,
        debug=False,
        enable_asserts=False,
        num_devices=N_CORES,
    )

    xt = nc.dram_tensor("xt", [P, KT, BC], F16, kind="ExternalInput")
    wd = nc.dram_tensor("wd", [P, BC], F16, kind="ExternalInput")
    ra = nc.dram_tensor("ra", [P, RT], F32, kind="ExternalInput")
    aal = nc.dram_tensor("aal", [KT, P, NR], F16, kind="ExternalInput")
    bal = nc.dram_tensor("bal", [RT, P, OUT], F16, kind="ExternalInput")
    mw = nc.dram_tensor("mw", [KT, P, OUT], F16, kind="ExternalInput")
    y = nc.dram_tensor("y", [BC, OUT], F16, kind="ExternalOutput")

    with tile.TileContext(nc) as tc:
        import contextlib

        loop_ctx = (
            tc.For_i(
                0,
                loop_n,
                1,
                staggered_reset=staggered,
                hint_engines=(
                    mybir.EngineType.PE,
                    mybir.EngineType.SP,
                    mybir.EngineType.Activation,
                    mybir.EngineType.DVE,
                    mybir.EngineType.Pool,
                ),
            )
            if loop_n is not None
            else contextlib.nullcontext()
        )
        with loop_ctx:
            with (
                tc.tile_pool(name="persist", bufs=1) as pp,
                tc.tile_pool(name="small", bufs=1) as sp,
                tc.tile_pool(name="mst", bufs=10) as mp,
                tc.tile_pool(name="bst", bufs=4) as bp,
                tc.tile_pool(name="ostg", bufs=3) as op_,
            ):
                ra_sb = sp.tile([P, RT], F32, name="ra_sb")
                wb_sb = sp.tile([P, BC], F16, name="wb_sb")
                xt_sb = pp.tile([P, KT, BC], F16, name="xt_sb")
                nc.sync.dma_start(out=xt_sb[:, 0:2, :], in_=xt.ap()[:, 0:2, :])
                nc.sync.dma_start(out=xt_sb[:, 2:8, :], in_=xt.ap()[:, 2:8, :])
                h_sb = pp.tile([P, RT, BC], F16, name="h_sb")
                # Phase H: H^T (all adapters), then mask -> h_sb (fp16).
                # k-outer with all 8 rank tiles accumulating in parallel PSUM
                # banks, so PE starts as soon as the first aal k-slice lands.
                psp = tc.alloc_tile_pool(name="psum", bufs=8, space="PSUM")
                with (
                    tc.tile_pool(name="ast", bufs=16) as ap_,
                    tc.tile_pool(name="maskp", bufs=8) as mkp,
                ):
                    msks = []
                    hpss = [
                        psp.tile([P, 512], F32, name=f"hps{rt}", tag="ps")[:, :BC]
                        for rt in range(RT)
                    ]
                    for k in range(KT):
                        if k == 6:
                            # mask inputs + masks: needed only at phase-H end
                            nc.gpsimd.dma_start(out=ra_sb[:], in_=ra.ap())
                            nc.gpsimd.dma_start(out=wb_sb[:], in_=wd.ap())
                            for rt in range(RT):
                                msk = mkp.tile(
                                    [P, BC], F16, name=f"msk{rt}", tag="msk"
                                )
                                nc.vector.tensor_scalar(
                                    out=msk[:],
                                    in0=wb_sb[:],
                                    scalar1=ra_sb[:, rt : rt + 1],
                                    scalar2=None,
                                    op0=mybir.AluOpType.is_equal,
                                )
                                msks.append(msk)
                        if k in (2, 10, 18):
                            kc = 8 * (k // 8 + 1)
                            nc.sync.dma_start(
                                out=xt_sb[:, kc : kc + 8, :],
                                in_=xt.ap()[:, kc : kc + 8, :],
                            )
                        at = ap_.tile([P, NR], F16, name="at", tag="at")
                        if k == 0:
                            # first tile split across both queues: halves the
                            # first matmul's data-arrival latency
                            nc.scalar.dma_start(
                                out=at[:, : NR // 2], in_=aal.ap()[0, :, : NR // 2]
                            )
                            nc.sync.dma_start(
                                out=at[:, NR // 2 :], in_=aal.ap()[0, :, NR // 2 :]
                            )
                        else:
                            eng = nc.scalar if k % 2 == 0 else nc.sync
                            eng.dma_start(out=at[:], in_=aal.ap()[k])
                        for rt in range(RT):
                            nc.tensor.matmul(
                                hpss[rt][:],
                                lhsT=at[:, rt * P : (rt + 1) * P],
                                rhs=xt_sb[:, k, :],
                                start=(k == 0),
                                stop=(k == KT - 1),
                            )
                    for rt in range(RT):
                        nc.vector.tensor_tensor(
                            out=h_sb[:, rt, :],
                            in0=hpss[rt][:],
                            in1=msks[rt][:],
                            op=mybir.AluOpType.mult,
                        )

                # Phase Y: y = x @ M + h_exp @ (2 * B_all), OUT in two halves.
                if True:
                    for h in range(NH):
                        ps = [
                            psp.tile([P, 512], F32, name=f"yps{h}_{j}", tag="ps")
                            for j in range(MB * NS)
                        ]
                        for k in range(KT):
                            mt = mp.tile([P, HW], F16, name="mt", tag="mt")
                            eng = nc.sync if k % 2 == 0 else nc.scalar
                            eng.dma_start(
                                out=mt[:], in_=mw.ap()[k, :, h * HW : (h + 1) * HW]
                            )
                            for mb in range(MB):
                                for ns in range(NS):
                                    nc.tensor.matmul(
                                        ps[mb * NS + ns][:],
                                        lhsT=xt_sb[:, k, mb * P : (mb + 1) * P],
                                        rhs=mt[:, ns * 512 : (ns + 1) * 512],
                                        start=(k == 0),
                                        stop=(k == KT - 1),
                                    )
                            # interleave one lora-B rank tile after every other
                            # M k-tile in the back half of the k-loop; spreads
                            # bt DMAs so the half-end has no load burst.
                            if k >= 8 and (k - 8) % 3 == 0:
                                rt = (k - 8) // 3
                                bt = bp.tile([P, HW], F16, name="bt", tag="bt")
                                eng = nc.sync if rt % 2 == 0 else nc.scalar
                                eng.dma_start(
                                    out=bt[:],
                                    in_=bal.ap()[rt, :, h * HW : (h + 1) * HW],
                                )
                                for mb in range(MB):
                                    for ns in range(NS):
                                        nc.tensor.matmul(
                                            ps[mb * NS + ns][:],
                                            lhsT=h_sb[:, rt, mb * P : (mb + 1) * P],
                                            rhs=bt[:, ns * 512 : (ns + 1) * 512],
                                            start=False,
                                            stop=False,
                                        )
                        for mb in range(MB):
                            ot = op_.tile([P, HW], F16, name="ot", tag="ot")
                            for ns in range(NS):
                                # split drains across DVE and ACT so the two
                                # banks' copies run in parallel at phase end
                                if ns % 2 == 0:
                                    nc.vector.tensor_copy(
                                        out=ot[:, ns * 512 : (ns + 1) * 512],
                                        in_=ps[mb * NS + ns][:],
                                    )
                                else:
                                    nc.scalar.copy(
                                        out=ot[:, ns * 512 : (ns + 1) * 512],
                                        in_=ps[mb * NS + ns][:],
                                    )
                                # ship each drained half as soon as it's ready
                                if ns == 1:
                                    nc.sync.dma_start(
                                        out=y.ap()[
                                            mb * P : (mb + 1) * P,
                                            h * HW : h * HW + 1024,
                                        ],
                                        in_=ot[:, :1024],
                                    )
                            nc.scalar.dma_start(
                                out=y.ap()[
                                    mb * P : (mb + 1) * P,
                                    h * HW + 1024 : (h + 1) * HW,
                                ],
                                in_=ot[:, 1024:],
                            )
                psp.release()

    nc.compile()
    return nc


def prep_inputs(x, wids, lora_A, lora_B, M):
    """Host-side sharding/layout prep. Returns per-core input maps."""
    x = np.asarray(x).reshape(B, IN).astype(np.float16, copy=False)
    wids = np.asarray(wids).reshape(B)
    lora_A = np.asarray(lora_A).astype(np.float16, copy=False)
    lora_B = np.asarray(lora_B).astype(np.float16, copy=False)
    M = np.asarray(M).astype(np.float16, copy=False)

    # [IN, NA*R]: column a*R+r is lora_A[a, :, r]
    aal_np = np.ascontiguousarray(
        lora_A.transpose(1, 0, 2).reshape(IN, NR).reshape(KT, P, NR)
    )
    # [NA*R, OUT] with the *2 output scale folded in (exact in fp16)
    bal_np = np.ascontiguousarray(
        (lora_B * np.float16(2.0)).reshape(NR, OUT).reshape(RT, P, OUT)
    )
    mw_np = np.ascontiguousarray(M.reshape(KT, P, OUT))
    ra_np = (
        (np.arange(RT)[None, :] * P + np.arange(P)[:, None]) // R
    ).astype(np.float32)

    in_maps = []
    for c in range(N_CORES):
        xs = x[c * BC : (c + 1) * BC]                      # [BC, IN]
        xt_np = np.ascontiguousarray(
            xs.T.reshape(KT, P, BC).transpose(1, 0, 2)
        )                                                  # [P, KT, BC]
        wd_np = np.ascontiguousarray(
            np.broadcast_to(
                wids[c * BC : (c + 1) * BC].astype(np.float16)[None, :], (P, BC)
            )
        )
        in_maps.append(
            {
                "xt": xt_np,
                "wd": wd_np,
                "ra": ra_np,
                "aal": aal_np,
                "bal": bal_np,
                "mw": mw_np,
            }
        )
    return in_maps


def kernel(x, wids, lora_A, lora_B, M):
    in_maps = prep_inputs(x, wids, lora_A, lora_B, M)
    nc = build_nc()
    res = run_bass_kernel_spmd(nc, in_maps, core_ids=list(range(N_CORES)))
    y = np.concatenate([res.results[c]["y"] for c in range(N_CORES)], axis=0)
    return y.reshape(B, 1, OUT)



# revision 9
# speedup vs baseline: 1.3339x; 1.3339x over previous
"""LoRA gather-BMM + dense GEMM kernel for Trainium2 (8 NeuronCores).

Computation (per the module semantics):
    A = lora_A[wids]; Bw = lora_B[wids]
    y = (x @ A) @ Bw * 2 + x @ M          # x: [B, 1, IN]

Distribution: data-parallel over batch, 256 samples per core.

Key trick: samples are SORTED by adapter id on the host. A contiguous
block of 128 sorted samples spans only a handful of adjacent adapters
(<= 8 for the target input distribution), so the per-block adapter
gather can be realized densely with just 128 stacked rank rows
(8 adapters x rank 16) instead of all 64 adapters' 1024 rows:

  h^T[s, j] = sum_i A_sel[i, s] * x^T[i, j]   (s = local rank slot)
  h^T      *= mask,  mask[s, j] = (wid[j] == base + s//16)
  y[j, :]   = x[j, :] @ M + h^T[:, j].T @ B_sel          (fp32 PSUM)

Per core: 2 sample blocks. The output is computed in 4 column
quarters of 1024; the LoRA-H matmuls ride inside quarter 0's k-loop
so their DMA/compute hides behind the big M GEMM. Outputs are
scattered back to original order on the host.

Falls back to a dense all-adapter variant (mask over 1024 rank rows)
if any sorted 128-sample window spans more than 8 adapters.
"""

import numpy as np

import concourse.bacc as bacc
import concourse.mybir as mybir
import concourse.tile as tile
from concourse.bass_utils import run_bass_kernel_spmd

B, IN, R, OUT, NA = 2048, 4096, 16, 4096, 64
N_CORES = 8
BC = B // N_CORES          # 256 samples per core
P = 128
KT = IN // P               # 32 contraction tiles over IN
NBLK = BC // P             # 2 sorted sample blocks per core
NQ = 4                     # output column quarters
QW = OUT // NQ             # 1024
NR = NA * R                # dense fallback: 1024 stacked rank rows
RT = NR // P

F16 = mybir.dt.float16
F32 = mybir.dt.float32


def build_nc(loop_n=None, staggered=False):
    nc = bacc.Bacc(
        "TRN2",
        target_bir_lowering=False,
        debug=False,
        enable_asserts=False,
        num_devices=N_CORES,
    )

    xt = nc.dram_tensor("xt", [P, KT, BC], F16, kind="ExternalInput")
    wd = nc.dram_tensor("wd", [P, BC], F16, kind="ExternalInput")
    ra = nc.dram_tensor("ra", [P, NBLK], F32, kind="ExternalInput")
    asel = nc.dram_tensor("asel", [P, KT, NBLK * P], F16, kind="ExternalInput")
    bsel = nc.dram_tensor("bsel", [NQ * NBLK, P, QW], F16, kind="ExternalInput")
    mw = nc.dram_tensor("mw", [NQ * KT, P, QW], F16, kind="ExternalInput")
    y = nc.dram_tensor("y", [BC, OUT], F16, kind="ExternalOutput")

    with tile.TileContext(nc) as tc:
        import contextlib

        loop_ctx = (
            tc.For_i(
                0,
                loop_n,
                1,
                staggered_reset=staggered,
                hint_engines=(
                    mybir.EngineType.PE,
                    mybir.EngineType.SP,
                    mybir.EngineType.Activation,
                    mybir.EngineType.DVE,
                    mybir.EngineType.Pool,
                ),
            )
            if loop_n is not None
            else contextlib.nullcontext()
        )
        with loop_ctx:
            with (
                tc.tile_pool(name="persist", bufs=1) as pp,
                tc.tile_pool(name="small", bufs=1) as sp,
                tc.tile_pool(name="mst", bufs=8) as mp,
                tc.tile_pool(name="bst", bufs=4) as bp,
                tc.tile_pool(name="ostg", bufs=4) as op_,
            ):
                xt_sb = pp.tile([P, KT, BC], F16, name="xt_sb")
                asel_sb = pp.tile([P, KT, NBLK * P], F16, name="asel_sb")
                h_sb = pp.tile([P, BC], F16, name="h_sb")
                wd_sb = sp.tile([P, BC], F16, name="wd_sb")
                ra_sb = sp.tile([P, NBLK], F32, name="ra_sb")
                msk = sp.tile([P, NBLK, P], F16, name="msk")

                # Upfront loads: x^T / A_sel head chunks on the sync HWDGE
                # ring so quarter 0 can start immediately; the rest stream in
                # during quarter 0 (staged below).  wids/rank-map via SWDGE.
                nc.sync.dma_start(out=xt_sb[:, 0:2, :], in_=xt.ap()[:, 0:2, :])
                nc.sync.dma_start(out=asel_sb[:, 0:2, :], in_=asel.ap()[:, 0:2, :])
                nc.sync.dma_start(out=xt_sb[:, 2:6, :], in_=xt.ap()[:, 2:6, :])
                nc.sync.dma_start(out=asel_sb[:, 2:6, :], in_=asel.ap()[:, 2:6, :])
                nc.gpsimd.dma_start(out=wd_sb[:], in_=wd.ap())
                nc.gpsimd.dma_start(out=ra_sb[:], in_=ra.ap())

                psy = tc.alloc_tile_pool(name="psy", bufs=6, space="PSUM")
                psh = tc.alloc_tile_pool(name="psh", bufs=2, space="PSUM")
                # one full PSUM bank per block: a start=True matmul clears the
                # has_written bits of its WHOLE bank, so the two blocks' H
                # accumulations must not share one.
                h_ps = [
                    psh.tile([P, 512], F32, name=f"h_ps{b}", tag="hs")
                    for b in range(NBLK)
                ]

                # sync-ring prefetch schedule inside quarter 0's k-loop:
                # alternate asel / xt chunks, each issued >=4 k-steps before
                # its first use so the in-order PE queue never waits on them.
                q0_pref = {
                    0: (asel_sb, asel, 6, 10),
                    2: (xt_sb, xt, 6, 12),
                    4: (asel_sb, asel, 10, 16),
                    6: (xt_sb, xt, 12, 18),
                    8: (asel_sb, asel, 16, 22),
                    10: (xt_sb, xt, 18, 24),
                    12: (asel_sb, asel, 22, 28),
                    14: (xt_sb, xt, 24, 32),
                    16: (asel_sb, asel, 28, 32),
                }

                for q in range(NQ):
                    ps = [
                        psy.tile([P, 512], F32, name=f"ps{q}_{t}", tag="ps")
                        for t in range(2 * NBLK)
                    ]
                    bts = [
                        bp.tile([P, QW], F16, name=f"bt{q}_{b}", tag="bt")
                        for b in range(NBLK)
                    ]
                    for k in range(KT):
                        if q == 0:
                            pref = q0_pref.get(k)
                            if pref is not None:
                                t_sb, t_dr, lo, hi = pref
                                nc.sync.dma_start(
                                    out=t_sb[:, lo:hi, :], in_=t_dr.ap()[:, lo:hi, :]
                                )
                            if k == 2:
                                # per-block gather masks (DVE, needs wd/ra)
                                for b in range(NBLK):
                                    nc.vector.tensor_scalar(
                                        out=msk[:, b, :],
                                        in0=wd_sb[:, b * P : (b + 1) * P],
                                        scalar1=ra_sb[:, b : b + 1],
                                        scalar2=None,
                                        op0=mybir.AluOpType.is_equal,
                                    )
                        if k == 1:
                            # LoRA-B panels for this quarter (SWDGE ring;
                            # needed at the b-matmuls ~25 us later)
                            for b in range(NBLK):
                                nc.gpsimd.dma_start(
                                    out=bts[b][:], in_=bsel.ap()[q * NBLK + b]
                                )
                        mt = mp.tile([P, QW], F16, name="mt", tag="mt")
                        nc.scalar.dma_start(out=mt[:], in_=mw.ap()[q * KT + k])
                        for b in range(NBLK):
                            for s in range(2):
                                nc.tensor.matmul(
                                    ps[b * 2 + s][:],
                                    lhsT=xt_sb[:, k, b * P : (b + 1) * P],
                                    rhs=mt[:, s * 512 : (s + 1) * 512],
                                    start=(k == 0),
                                    stop=(k == KT - 1 and q > 0),
                                )
                        if q == 0:
                            # LoRA-H: h^T accumulation rides the q0 k-loop
                            for b in range(NBLK):
                                nc.tensor.matmul(
                                    h_ps[b][:, 0:P],
                                    lhsT=asel_sb[:, k, b * P : (b + 1) * P],
                                    rhs=xt_sb[:, k, b * P : (b + 1) * P],
                                    start=(k == 0),
                                    stop=(k == KT - 1),
                                )
                        if q > 0 and k == 8:
                            # LoRA-B contribution (h ready since quarter 0)
                            for b in range(NBLK):
                                for s in range(2):
                                    nc.tensor.matmul(
                                        ps[b * 2 + s][:],
                                        lhsT=h_sb[:, b * P : (b + 1) * P],
                                        rhs=bts[b][:, s * 512 : (s + 1) * 512],
                                        start=False,
                                        stop=False,
                                    )
                    if q == 0:
                        # realize the gather: h = h_dense * mask  (fp16)
                        for b in range(NBLK):
                            nc.vector.tensor_tensor(
                                out=h_sb[:, b * P : (b + 1) * P],
                                in0=h_ps[b][:, 0:P],
                                in1=msk[:, b, :],
                                op=mybir.AluOpType.mult,
                            )
                        for b in range(NBLK):
                            for s in range(2):
                                nc.tensor.matmul(
                                    ps[b * 2 + s][:],
                                    lhsT=h_sb[:, b * P : (b + 1) * P],
                                    rhs=bts[b][:, s * 512 : (s + 1) * 512],
                                    start=False,
                                    stop=True,
                                )
                    # drain: split PSUM->SBUF copies across DVE and ACT,
                    # ship each block's quarter as soon as it is staged
                    for b in range(NBLK):
                        ot = op_.tile([P, QW], F16, name="ot", tag="ot")
                        nc.vector.tensor_copy(
                            out=ot[:, 0:512], in_=ps[b * 2 + 0][:]
                        )
                        nc.scalar.copy(out=ot[:, 512:1024], in_=ps[b * 2 + 1][:])
                        nc.sync.dma_start(
                            out=y.ap()[
                                b * P : (b + 1) * P, q * QW : (q + 1) * QW
                            ],
                            in_=ot[:],
                        )
                psh.release()
                psy.release()

    nc.compile()
    return nc


def _sort_blocks(wids):
    """Sort samples by adapter; return (order, bases) or None if any
    128-sample window spans more than 8 adapters."""
    order = np.argsort(wids, kind="stable")
    ws = wids[order]
    nblk = B // P
    starts = np.arange(nblk) * P
    bases = np.minimum(ws[starts], NA - 8).astype(np.int64)
    if np.any(ws[starts + P - 1] > bases + 7):
        return None
    return order, bases


def prep_inputs(x, wids, lora_A, lora_B, M):
    """Host-side sharding/layout prep. Returns per-core input maps."""
    x = np.asarray(x).reshape(B, IN).astype(np.float16, copy=False)
    wids = np.asarray(wids).reshape(B)
    lora_A = np.asarray(lora_A).astype(np.float16, copy=False)
    lora_B = np.asarray(lora_B).astype(np.float16, copy=False)
    M = np.asarray(M).astype(np.float16, copy=False)

    sb = _sort_blocks(wids)
    assert sb is not None, "sorted windows span >8 adapters; need dense path"
    order, bases = sb
    xs = x[order]
    ws = wids[order]

    # [IN, NA*R]: column a*R+r is lora_A[a, :, r]
    A_T = np.ascontiguousarray(lora_A.transpose(1, 0, 2).reshape(IN, NR))
    # [NA*R, OUT] with the *2 output scale folded in (exact in fp16)
    B_all = (lora_B * np.float16(2.0)).reshape(NR, OUT)
    # [NQ*KT, P, QW]: quarter-major M tiles, contiguous per (q, k)
    mw_np = np.ascontiguousarray(
        M.reshape(KT, P, NQ, QW).transpose(2, 0, 1, 3).reshape(NQ * KT, P, QW)
    )

    rr = np.arange(P) // R
    in_maps = []
    for c in range(N_CORES):
        rows = slice(c * BC, (c + 1) * BC)
        xt_np = np.ascontiguousarray(
            xs[rows].T.reshape(KT, P, BC).transpose(1, 0, 2)
        )
        wd_np = np.ascontiguousarray(
            np.broadcast_to(ws[rows].astype(np.float16)[None, :], (P, BC))
        )
        blk = [NBLK * c + j for j in range(NBLK)]
        ra_np = np.ascontiguousarray(
            np.stack([bases[t] + rr for t in blk], axis=1).astype(np.float32)
        )
        # A_sel: [P, KT, NBLK*P]; per block the 8 adapters' A columns
        a_list = [
            A_T[:, bases[t] * R : bases[t] * R + P].reshape(KT, P, P)
            for t in blk
        ]
        asel_np = np.ascontiguousarray(
            np.stack(a_list, axis=2).transpose(1, 0, 2, 3).reshape(P, KT, NBLK * P)
        )
        # B_sel: [NQ*NBLK, P, QW]
        b_list = [
            B_all[bases[t] * R : bases[t] * R + P].reshape(P, NQ, QW)
            for t in blk
        ]
        bsel_np = np.ascontiguousarray(
            np.stack(b_list, axis=0)
            .transpose(2, 0, 1, 3)
            .reshape(NQ * NBLK, P, QW)
        )
        in_maps.append(
            {
                "xt": xt_np,
                "wd": wd_np,
                "ra": ra_np,
                "asel": asel_np,
                "bsel": bsel_np,
                "mw": mw_np,
            }
        )
    return in_maps


def kernel(x, wids, lora_A, lora_B, M):
    wids_np = np.asarray(wids).reshape(B)
    if _sort_blocks(wids_np) is None:
        return _kernel_dense(x, wids, lora_A, lora_B, M)
    in_maps = prep_inputs(x, wids, lora_A, lora_B, M)
    nc = build_nc()
    res = run_bass_kernel_spmd(nc, in_maps, core_ids=list(range(N_CORES)))
    ys = np.concatenate([res.results[c]["y"] for c in range(N_CORES)], axis=0)
    order, _ = _sort_blocks(wids_np)
    yf = np.empty_like(ys)
    yf[order] = ys
    return yf.reshape(B, 1, OUT)


# ---------------------------------------------------------------------------
# Dense fallback (all-adapter mask over 1024 rank rows) — used only if the
# sorted-window precondition fails for an unexpected wids distribution.
# ---------------------------------------------------------------------------

KT_D = KT
RT_D = RT
NH_D = 2
HW_D = OUT // NH_D
NS_D = HW_D // 512
MB_D = BC // P


def build_nc_dense(loop_n=None, staggered=False):
    nc = bacc.Bacc(
        "TRN2",
        target_bir_lowering=False,
        debug=False,
        enable_asserts=False,
        num_devices=N_CORES,
    )

    xt = nc.dram_tensor("xt", [P, KT_D, BC], F16, kind="ExternalInput")
    wdt = nc.dram_tensor("wd", [P, BC], F16, kind="ExternalInput")
    rat = nc.dram_tensor("ra", [P, RT_D], F32, kind="ExternalInput")
    aal = nc.dram_tensor("aal", [KT_D, P, NR], F16, kind="ExternalInput")
    bal = nc.dram_tensor("bal", [RT_D, P, OUT], F16, kind="ExternalInput")
    mw = nc.dram_tensor("mw", [KT_D, P, OUT], F16, kind="ExternalInput")
    y = nc.dram_tensor("y", [BC, OUT], F16, kind="ExternalOutput")

    with tile.TileContext(nc) as tc:
        with (
            tc.tile_pool(name="persist", bufs=1) as pp,
            tc.tile_pool(name="small", bufs=1) as sp,
            tc.tile_pool(name="mst", bufs=10) as mp,
            tc.tile_pool(name="bst", bufs=4) as bp,
            tc.tile_pool(name="ostg", bufs=3) as op_,
        ):
            ra_sb = sp.tile([P, RT_D], F32, name="ra_sb")
            wb_sb = sp.tile([P, BC], F16, name="wb_sb")
            xt_sb = pp.tile([P, KT_D, BC], F16, name="xt_sb")
            nc.sync.dma_start(out=xt_sb[:, 0:2, :], in_=xt.ap()[:, 0:2, :])
            nc.sync.dma_start(out=xt_sb[:, 2:8, :], in_=xt.ap()[:, 2:8, :])
            h_sb = pp.tile([P, RT_D, BC], F16, name="h_sb")
            psp = tc.alloc_tile_pool(name="psum", bufs=8, space="PSUM")
            with (
                tc.tile_pool(name="ast", bufs=16) as ap_,
                tc.tile_pool(name="maskp", bufs=8) as mkp,
            ):
                msks = []
                hpss = [
                    psp.tile([P, 512], F32, name=f"hps{rt}", tag="ps")[:, :BC]
                    for rt in range(RT_D)
                ]
                for k in range(KT_D):
                    if k == 6:
                        nc.gpsimd.dma_start(out=ra_sb[:], in_=rat.ap())
                        nc.gpsimd.dma_start(out=wb_sb[:], in_=wdt.ap())
                        for rt in range(RT_D):
                            msk = mkp.tile([P, BC], F16, name=f"msk{rt}", tag="msk")
                            nc.vector.tensor_scalar(
                                out=msk[:],
                                in0=wb_sb[:],
                                scalar1=ra_sb[:, rt : rt + 1],
                                scalar2=None,
                                op0=mybir.AluOpType.is_equal,
                            )
                            msks.append(msk)
                    if k in (2, 10, 18):
                        kc = 8 * (k // 8 + 1)
                        nc.sync.dma_start(
                            out=xt_sb[:, kc : kc + 8, :],
                            in_=xt.ap()[:, kc : kc + 8, :],
                        )
                    at = ap_.tile([P, NR], F16, name="at", tag="at")
                    if k == 0:
                        nc.scalar.dma_start(
                            out=at[:, : NR // 2], in_=aal.ap()[0, :, : NR // 2]
                        )
                        nc.sync.dma_start(
                            out=at[:, NR // 2 :], in_=aal.ap()[0, :, NR // 2 :]
                        )
                    else:
                        eng = nc.scalar if k % 2 == 0 else nc.sync
                        eng.dma_start(out=at[:], in_=aal.ap()[k])
                    for rt in range(RT_D):
                        nc.tensor.matmul(
                            hpss[rt][:],
                            lhsT=at[:, rt * P : (rt + 1) * P],
                            rhs=xt_sb[:, k, :],
                            start=(k == 0),
                            stop=(k == KT_D - 1),
                        )
                for rt in range(RT_D):
                    nc.vector.tensor_tensor(
                        out=h_sb[:, rt, :],
                        in0=hpss[rt][:],
                        in1=msks[rt][:],
                        op=mybir.AluOpType.mult,
                    )

            for h in range(NH_D):
                ps = [
                    psp.tile([P, 512], F32, name=f"yps{h}_{j}", tag="ps")
                    for j in range(MB_D * NS_D)
                ]
                for k in range(KT_D):
                    mt = mp.tile([P, HW_D], F16, name="mt", tag="mt")
                    eng = nc.sync if k % 2 == 0 else nc.scalar
                    eng.dma_start(
                        out=mt[:], in_=mw.ap()[k, :, h * HW_D : (h + 1) * HW_D]
                    )
                    for mb in range(MB_D):
                        for ns in range(NS_D):
                            nc.tensor.matmul(
                                ps[mb * NS_D + ns][:],
                                lhsT=xt_sb[:, k, mb * P : (mb + 1) * P],
                                rhs=mt[:, ns * 512 : (ns + 1) * 512],
                                start=(k == 0),
                                stop=(k == KT_D - 1),
                            )
                    if k >= 8 and (k - 8) % 3 == 0:
                        rt = (k - 8) // 3
                        bt = bp.tile([P, HW_D], F16, name="bt", tag="bt")
                        eng = nc.sync if rt % 2 == 0 else nc.scalar
                        eng.dma_start(
                            out=bt[:],
                            in_=bal.ap()[rt, :, h * HW_D : (h + 1) * HW_D],
                        )
                        for mb in range(MB_D):
                            for ns in range(NS_D):
                                nc.tensor.matmul(
                                    ps[mb * NS_D + ns][:],
                                    lhsT=h_sb[:, rt, mb * P : (mb + 1) * P],
                                    rhs=bt[:, ns * 512 : (ns + 1) * 512],
                                    start=False,
                                    stop=False,
                                )
                for mb in range(MB_D):
                    ot = op_.tile([P, HW_D], F16, name="ot", tag="ot")
                    for ns in range(NS_D):
                        if ns % 2 == 0:
                            nc.vector.tensor_copy(
                                out=ot[:, ns * 512 : (ns + 1) * 512],
                                in_=ps[mb * NS_D + ns][:],
                            )
                        else:
                            nc.scalar.copy(
                                out=ot[:, ns * 512 : (ns + 1) * 512],
                                in_=ps[mb * NS_D + ns][:],
                            )
                        if ns == 1:
                            nc.sync.dma_start(
                                out=y.ap()[
                                    mb * P : (mb + 1) * P,
                                    h * HW_D : h * HW_D + 1024,
                                ],
                                in_=ot[:, :1024],
                            )
                    nc.scalar.dma_start(
                        out=y.ap()[
                            mb * P : (mb + 1) * P,
                            h * HW_D + 1024 : (h + 1) * HW_D,
                        ],
                        in_=ot[:, 1024:],
                    )
            psp.release()

    nc.compile()
    return nc


def _kernel_dense(x, wids, lora_A, lora_B, M):
    x = np.asarray(x).reshape(B, IN).astype(np.float16, copy=False)
    wids = np.asarray(wids).reshape(B)
    lora_A = np.asarray(lora_A).astype(np.float16, copy=False)
    lora_B = np.asarray(lora_B).astype(np.float16, copy=False)
    M = np.asarray(M).astype(np.float16, copy=False)

    aal_np = np.ascontiguousarray(
        lora_A.transpose(1, 0, 2).reshape(IN, NR).reshape(KT_D, P, NR)
    )
    bal_np = np.ascontiguousarray(
        (lora_B * np.float16(2.0)).reshape(NR, OUT).reshape(RT_D, P, OUT)
    )
    mw_np = np.ascontiguousarray(M.reshape(KT_D, P, OUT))
    ra_np = (
        (np.arange(RT_D)[None, :] * P + np.arange(P)[:, None]) // R
    ).astype(np.float32)

    in_maps = []
    for c in range(N_CORES):
        xs = x[c * BC : (c + 1) * BC]
        xt_np = np.ascontiguousarray(xs.T.reshape(KT_D, P, BC).transpose(1, 0, 2))
        wd_np = np.ascontiguousarray(
            np.broadcast_to(
                wids[c * BC : (c + 1) * BC].astype(np.float16)[None, :], (P, BC)
            )
        )
        in_maps.append(
            {
                "xt": xt_np,
                "wd": wd_np,
                "ra": ra_np,
                "aal": aal_np,
                "bal": bal_np,
                "mw": mw_np,
            }
        )
    nc = build_nc_dense()
    res = run_bass_kernel_spmd(nc, in_maps, core_ids=list(range(N_CORES)))
    y = np.concatenate([res.results[c]["y"] for c in range(N_CORES)], axis=0)
    return y.reshape(B, 1, OUT)


# revision 12
# speedup vs baseline: 1.3482x; 1.0107x over previous
"""LoRA gather-BMM + dense GEMM kernel for Trainium2 (8 NeuronCores).

Computation (per the module semantics):
    A = lora_A[wids]; Bw = lora_B[wids]
    y = (x @ A) @ Bw * 2 + x @ M          # x: [B, 1, IN]

Distribution: data-parallel over batch, 256 samples per core.

Key trick: samples are SORTED by adapter id on the host. A contiguous
block of 128 sorted samples spans only a handful of adjacent adapters
(<= 8 for the target input distribution), so the per-block adapter
gather can be realized densely with just 128 stacked rank rows
(8 adapters x rank 16) instead of all 64 adapters' 1024 rows:

  h^T[s, j] = sum_i A_sel[i, s] * x^T[i, j]   (s = local rank slot)
  h^T      *= mask,  mask[s, j] = (wid[j] == base + s//16)
  y[j, :]   = x[j, :] @ M + h^T[:, j].T @ B_sel          (fp32 PSUM)

Per core: 2 sample blocks. The output is computed in 4 column
quarters of 1024; the LoRA-H matmuls ride inside quarter 0's k-loop
so their DMA/compute hides behind the big M GEMM. Outputs are
scattered back to original order on the host.

Falls back to a dense all-adapter variant (mask over 1024 rank rows)
if any sorted 128-sample window spans more than 8 adapters.
"""

import numpy as np

import concourse.bacc as bacc
import concourse.mybir as mybir
import concourse.tile as tile
from concourse.bass_utils import run_bass_kernel_spmd

B, IN, R, OUT, NA = 2048, 4096, 16, 4096, 64
N_CORES = 8
BC = B // N_CORES          # 256 samples per core
P = 128
KT = IN // P               # 32 contraction tiles over IN
NBLK = BC // P             # 2 sorted sample blocks per core
NQ = 4                     # output column quarters
QW = OUT // NQ             # 1024
NR = NA * R                # dense fallback: 1024 stacked rank rows
RT = NR // P

F16 = mybir.dt.float16
F32 = mybir.dt.float32


def build_nc(loop_n=None, staggered=False):
    nc = bacc.Bacc(
        "TRN2",
        target_bir_lowering=False,
        debug=False,
        enable_asserts=False,
        num_devices=N_CORES,
    )

    xt = nc.dram_tensor("xt", [P, KT, BC], F16, kind="ExternalInput")
    wd = nc.dram_tensor("wd", [P, BC], F16, kind="ExternalInput")
    ra = nc.dram_tensor("ra", [P, NBLK], F32, kind="ExternalInput")
    asel = nc.dram_tensor("asel", [P, KT, NBLK * P], F16, kind="ExternalInput")
    bsel = nc.dram_tensor("bsel", [NQ * NBLK, P, QW], F16, kind="ExternalInput")
    mw = nc.dram_tensor("mw", [NQ * KT, P, QW], F16, kind="ExternalInput")
    y = nc.dram_tensor("y", [BC, OUT], F16, kind="ExternalOutput")

    with tile.TileContext(nc) as tc:
        import contextlib

        loop_ctx = (
            tc.For_i(
                0,
                loop_n,
                1,
                staggered_reset=staggered,
                hint_engines=(
                    mybir.EngineType.PE,
                    mybir.EngineType.SP,
                    mybir.EngineType.Activation,
                    mybir.EngineType.DVE,
                    mybir.EngineType.Pool,
                ),
            )
            if loop_n is not None
            else contextlib.nullcontext()
        )
        with loop_ctx:
            with (
                tc.tile_pool(name="persist", bufs=1) as pp,
                tc.tile_pool(name="small", bufs=1) as sp,
                tc.tile_pool(name="mst", bufs=12) as mp,
                tc.tile_pool(name="bst", bufs=4) as bp,
                tc.tile_pool(name="ostg", bufs=4) as op_,
            ):
                xt_sb = pp.tile([P, KT, BC], F16, name="xt_sb")
                asel_sb = pp.tile([P, KT, NBLK * P], F16, name="asel_sb")
                h_sb = pp.tile([P, BC], F16, name="h_sb")
                wd_sb = sp.tile([P, BC], F16, name="wd_sb")
                ra_sb = sp.tile([P, NBLK], F32, name="ra_sb")
                msk = sp.tile([P, NBLK, P], F16, name="msk")

                # Upfront loads: x^T / A_sel head chunks on the sync HWDGE
                # ring so quarter 0 can start immediately; the rest stream in
                # during quarter 0 (staged below).  wids/rank-map via SWDGE.
                nc.sync.dma_start(out=xt_sb[:, 0:2, :], in_=xt.ap()[:, 0:2, :])
                nc.sync.dma_start(out=asel_sb[:, 0:2, :], in_=asel.ap()[:, 0:2, :])
                nc.sync.dma_start(out=xt_sb[:, 2:6, :], in_=xt.ap()[:, 2:6, :])
                nc.sync.dma_start(out=asel_sb[:, 2:6, :], in_=asel.ap()[:, 2:6, :])
                nc.gpsimd.dma_start(out=wd_sb[:], in_=wd.ap())
                nc.gpsimd.dma_start(out=ra_sb[:], in_=ra.ap())

                psy = tc.alloc_tile_pool(name="psy", bufs=6, space="PSUM")
                psh = tc.alloc_tile_pool(name="psh", bufs=2, space="PSUM")
                # one full PSUM bank per block: a start=True matmul clears the
                # has_written bits of its WHOLE bank, so the two blocks' H
                # accumulations must not share one.
                h_ps = [
                    psh.tile([P, 512], F32, name=f"h_ps{b}", tag="hs")
                    for b in range(NBLK)
                ]

                # sync-ring prefetch schedule inside quarter 0's k-loop:
                # alternate asel / xt chunks, each issued >=4 k-steps before
                # its first use so the in-order PE queue never waits on them.
                q0_pref = {
                    0: (asel_sb, asel, 6, 10),
                    2: (xt_sb, xt, 6, 12),
                    4: (asel_sb, asel, 10, 16),
                    6: (xt_sb, xt, 12, 18),
                    8: (asel_sb, asel, 16, 22),
                    10: (xt_sb, xt, 18, 24),
                    12: (asel_sb, asel, 22, 28),
                    14: (xt_sb, xt, 24, 32),
                    16: (asel_sb, asel, 28, 32),
                }

                for q in range(NQ):
                    ps = [
                        psy.tile([P, 512], F32, name=f"ps{q}_{t}", tag="ps")
                        for t in range(2 * NBLK)
                    ]
                    bts = [
                        bp.tile([P, QW], F16, name=f"bt{q}_{b}", tag="bt")
                        for b in range(NBLK)
                    ]
                    for k in range(KT):
                        if q == 0:
                            pref = q0_pref.get(k)
                            if pref is not None:
                                t_sb, t_dr, lo, hi = pref
                                nc.sync.dma_start(
                                    out=t_sb[:, lo:hi, :], in_=t_dr.ap()[:, lo:hi, :]
                                )
                            if k == 2:
                                # per-block gather masks (DVE, needs wd/ra)
                                for b in range(NBLK):
                                    nc.vector.tensor_scalar(
                                        out=msk[:, b, :],
                                        in0=wd_sb[:, b * P : (b + 1) * P],
                                        scalar1=ra_sb[:, b : b + 1],
                                        scalar2=None,
                                        op0=mybir.AluOpType.is_equal,
                                    )
                        if k == 1:
                            # LoRA-B panels for this quarter (sync ring;
                            # needed at the b-matmuls ~25 us later)
                            for b in range(NBLK):
                                nc.sync.dma_start(
                                    out=bts[b][:], in_=bsel.ap()[q * NBLK + b]
                                )
                        mt = mp.tile([P, QW], F16, name="mt", tag="mt")
                        nc.scalar.dma_start(out=mt[:], in_=mw.ap()[q * KT + k])
                        for b in range(NBLK):
                            for s in range(2):
                                nc.tensor.matmul(
                                    ps[b * 2 + s][:],
                                    lhsT=xt_sb[:, k, b * P : (b + 1) * P],
                                    rhs=mt[:, s * 512 : (s + 1) * 512],
                                    start=(k == 0),
                                    stop=(k == KT - 1 and q > 0),
                                )
                        if q == 0:
                            # LoRA-H: h^T accumulation rides the q0 k-loop
                            for b in range(NBLK):
                                nc.tensor.matmul(
                                    h_ps[b][:, 0:P],
                                    lhsT=asel_sb[:, k, b * P : (b + 1) * P],
                                    rhs=xt_sb[:, k, b * P : (b + 1) * P],
                                    start=(k == 0),
                                    stop=(k == KT - 1),
                                )
                        if q > 0 and k == 8:
                            # LoRA-B contribution (h ready since quarter 0)
                            for b in range(NBLK):
                                for s in range(2):
                                    nc.tensor.matmul(
                                        ps[b * 2 + s][:],
                                        lhsT=h_sb[:, b * P : (b + 1) * P],
                                        rhs=bts[b][:, s * 512 : (s + 1) * 512],
                                        start=False,
                                        stop=False,
                                    )
                    if q == 0:
                        # realize the gather: h = h_dense * mask  (fp16)
                        for b in range(NBLK):
                            nc.vector.tensor_tensor(
                                out=h_sb[:, b * P : (b + 1) * P],
                                in0=h_ps[b][:, 0:P],
                                in1=msk[:, b, :],
                                op=mybir.AluOpType.mult,
                            )
                        for b in range(NBLK):
                            for s in range(2):
                                nc.tensor.matmul(
                                    ps[b * 2 + s][:],
                                    lhsT=h_sb[:, b * P : (b + 1) * P],
                                    rhs=bts[b][:, s * 512 : (s + 1) * 512],
                                    start=False,
                                    stop=True,
                                )
                    # drain: ALL PSUM->SBUF copies on DVE.  The ACT engine
                    # must stay a pure mw-DMA issuer: a drain copy on ACT
                    # carries a sem-wait that blocks every later mw dma_start
                    # on that ring, starving the PE at each quarter boundary.
                    for b in range(NBLK):
                        ot = op_.tile([P, QW], F16, name="ot", tag="ot")
                        nc.vector.tensor_copy(
                            out=ot[:, 0:512], in_=ps[b * 2 + 0][:]
                        )
                        nc.vector.tensor_copy(
                            out=ot[:, 512:1024], in_=ps[b * 2 + 1][:]
                        )
                        nc.sync.dma_start(
                            out=y.ap()[
                                b * P : (b + 1) * P, q * QW : (q + 1) * QW
                            ],
                            in_=ot[:],
                        )
                psh.release()
                psy.release()

    nc.compile()
    return nc


def _sort_blocks(wids):
    """Sort samples by adapter; return (order, bases) or None if any
    128-sample window spans more than 8 adapters."""
    order = np.argsort(wids, kind="stable")
    ws = wids[order]
    nblk = B // P
    starts = np.arange(nblk) * P
    bases = np.minimum(ws[starts], NA - 8).astype(np.int64)
    if np.any(ws[starts + P - 1] > bases + 7):
        return None
    return order, bases


def prep_inputs(x, wids, lora_A, lora_B, M):
    """Host-side sharding/layout prep. Returns per-core input maps."""
    x = np.asarray(x).reshape(B, IN).astype(np.float16, copy=False)
    wids = np.asarray(wids).reshape(B)
    lora_A = np.asarray(lora_A).astype(np.float16, copy=False)
    lora_B = np.asarray(lora_B).astype(np.float16, copy=False)
    M = np.asarray(M).astype(np.float16, copy=False)

    sb = _sort_blocks(wids)
    assert sb is not None, "sorted windows span >8 adapters; need dense path"
    order, bases = sb
    xs = x[order]
    ws = wids[order]

    # [IN, NA*R]: column a*R+r is lora_A[a, :, r]
    A_T = np.ascontiguousarray(lora_A.transpose(1, 0, 2).reshape(IN, NR))
    # [NA*R, OUT] with the *2 output scale folded in (exact in fp16)
    B_all = (lora_B * np.float16(2.0)).reshape(NR, OUT)
    # [NQ*KT, P, QW]: quarter-major M tiles, contiguous per (q, k)
    mw_np = np.ascontiguousarray(
        M.reshape(KT, P, NQ, QW).transpose(2, 0, 1, 3).reshape(NQ * KT, P, QW)
    )

    rr = np.arange(P) // R
    in_maps = []
    for c in range(N_CORES):
        rows = slice(c * BC, (c + 1) * BC)
        xt_np = np.ascontiguousarray(
            xs[rows].T.reshape(KT, P, BC).transpose(1, 0, 2)
        )
        wd_np = np.ascontiguousarray(
            np.broadcast_to(ws[rows].astype(np.float16)[None, :], (P, BC))
        )
        blk = [NBLK * c + j for j in range(NBLK)]
        ra_np = np.ascontiguousarray(
            np.stack([bases[t] + rr for t in blk], axis=1).astype(np.float32)
        )
        # A_sel: [P, KT, NBLK*P]; per block the 8 adapters' A columns
        a_list = [
            A_T[:, bases[t] * R : bases[t] * R + P].reshape(KT, P, P)
            for t in blk
        ]
        asel_np = np.ascontiguousarray(
            np.stack(a_list, axis=2).transpose(1, 0, 2, 3).reshape(P, KT, NBLK * P)
        )
        # B_sel: [NQ*NBLK, P, QW]
        b_list = [
            B_all[bases[t] * R : bases[t] * R + P].reshape(P, NQ, QW)
            for t in blk
        ]
        bsel_np = np.ascontiguousarray(
            np.stack(b_list, axis=0)
            .transpose(2, 0, 1, 3)
            .reshape(NQ * NBLK, P, QW)
        )
        in_maps.append(
            {
                "xt": xt_np,
                "wd": wd_np,
                "ra": ra_np,
                "asel": asel_np,
                "bsel": bsel_np,
                "mw": mw_np,
            }
        )
    return in_maps


def kernel(x, wids, lora_A, lora_B, M):
    wids_np = np.asarray(wids).reshape(B)
    if _sort_blocks(wids_np) is None:
        return _kernel_dense(x, wids, lora_A, lora_B, M)
    in_maps = prep_inputs(x, wids, lora_A, lora_B, M)
    nc = build_nc()
    res = run_bass_kernel_spmd(nc, in_maps, core_ids=list(range(N_CORES)))
    ys = np.concatenate([res.results[c]["y"] for c in range(N_CORES)], axis=0)
    order, _ = _sort_blocks(wids_np)
    yf = np.empty_like(ys)
    yf[order] = ys
    return yf.reshape(B, 1, OUT)


# ---------------------------------------------------------------------------
# Dense fallback (all-adapter mask over 1024 rank rows) — used only if the
# sorted-window precondition fails for an unexpected wids distribution.
# ---------------------------------------------------------------------------

KT_D = KT
RT_D = RT
NH_D = 2
HW_D = OUT // NH_D
NS_D = HW_D // 512
MB_D = BC // P


def build_nc_dense(loop_n=None, staggered=False):
    nc = bacc.Bacc(
        "TRN2",
        target_bir_lowering=False,
        debug=False,
        enable_asserts=False,
        num_devices=N_CORES,
    )

    xt = nc.dram_tensor("xt", [P, KT_D, BC], F16, kind="ExternalInput")
    wdt = nc.dram_tensor("wd", [P, BC], F16, kind="ExternalInput")
    rat = nc.dram_tensor("ra", [P, RT_D], F32, kind="ExternalInput")
    aal = nc.dram_tensor("aal", [KT_D, P, NR], F16, kind="ExternalInput")
    bal = nc.dram_tensor("bal", [RT_D, P, OUT], F16, kind="ExternalInput")
    mw = nc.dram_tensor("mw", [KT_D, P, OUT], F16, kind="ExternalInput")
    y = nc.dram_tensor("y", [BC, OUT], F16, kind="ExternalOutput")

    with tile.TileContext(nc) as tc:
        with (
            tc.tile_pool(name="persist", bufs=1) as pp,
            tc.tile_pool(name="small", bufs=1) as sp,
            tc.tile_pool(name="mst", bufs=10) as mp,
            tc.tile_pool(name="bst", bufs=4) as bp,
            tc.tile_pool(name="ostg", bufs=3) as op_,
        ):
            ra_sb = sp.tile([P, RT_D], F32, name="ra_sb")
            wb_sb = sp.tile([P, BC], F16, name="wb_sb")
            xt_sb = pp.tile([P, KT_D, BC], F16, name="xt_sb")
            nc.sync.dma_start(out=xt_sb[:, 0:2, :], in_=xt.ap()[:, 0:2, :])
            nc.sync.dma_start(out=xt_sb[:, 2:8, :], in_=xt.ap()[:, 2:8, :])
            h_sb = pp.tile([P, RT_D, BC], F16, name="h_sb")
            psp = tc.alloc_tile_pool(name="psum", bufs=8, space="PSUM")
            with (
                tc.tile_pool(name="ast", bufs=16) as ap_,
                tc.tile_pool(name="maskp", bufs=8) as mkp,
            ):
                msks = []
                hpss = [
                    psp.tile([P, 512], F32, name=f"hps{rt}", tag="ps")[:, :BC]
                    for rt in range(RT_D)
                ]
                for k in range(KT_D):
                    if k == 6:
                        nc.gpsimd.dma_start(out=ra_sb[:], in_=rat.ap())
                        nc.gpsimd.dma_start(out=wb_sb[:], in_=wdt.ap())
                        for rt in range(RT_D):
                            msk = mkp.tile([P, BC], F16, name=f"msk{rt}", tag="msk")
                            nc.vector.tensor_scalar(
                                out=msk[:],
                                in0=wb_sb[:],
                                scalar1=ra_sb[:, rt : rt + 1],
                                scalar2=None,
                                op0=mybir.AluOpType.is_equal,
                            )
                            msks.append(msk)
                    if k in (2, 10, 18):
                        kc = 8 * (k // 8 + 1)
                        nc.sync.dma_start(
                            out=xt_sb[:, kc : kc + 8, :],
                            in_=xt.ap()[:, kc : kc + 8, :],
                        )
                    at = ap_.tile([P, NR], F16, name="at", tag="at")
                    if k == 0:
                        nc.scalar.dma_start(
                            out=at[:, : NR // 2], in_=aal.ap()[0, :, : NR // 2]
                        )
                        nc.sync.dma_start(
                            out=at[:, NR // 2 :], in_=aal.ap()[0, :, NR // 2 :]
                        )
                    else:
                        eng = nc.scalar if k % 2 == 0 else nc.sync
                        eng.dma_start(out=at[:], in_=aal.ap()[k])
                    for rt in range(RT_D):
                        nc.tensor.matmul(
                            hpss[rt][:],
                            lhsT=at[:, rt * P : (rt + 1) * P],
                            rhs=xt_sb[:, k, :],
                            start=(k == 0),
                            stop=(k == KT_D - 1),
                        )
                for rt in range(RT_D):
                    nc.vector.tensor_tensor(
                        out=h_sb[:, rt, :],
                        in0=hpss[rt][:],
                        in1=msks[rt][:],
                        op=mybir.AluOpType.mult,
                    )

            for h in range(NH_D):
                ps = [
                    psp.tile([P, 512], F32, name=f"yps{h}_{j}", tag="ps")
                    for j in range(MB_D * NS_D)
                ]
                for k in range(KT_D):
                    mt = mp.tile([P, HW_D], F16, name="mt", tag="mt")
                    eng = nc.sync if k % 2 == 0 else nc.scalar
                    eng.dma_start(
                        out=mt[:], in_=mw.ap()[k, :, h * HW_D : (h + 1) * HW_D]
                    )
                    for mb in range(MB_D):
                        for ns in range(NS_D):
                            nc.tensor.matmul(
                                ps[mb * NS_D + ns][:],
                                lhsT=xt_sb[:, k, mb * P : (mb + 1) * P],
                                rhs=mt[:, ns * 512 : (ns + 1) * 512],
                                start=(k == 0),
                                stop=(k == KT_D - 1),
                            )
                    if k >= 8 and (k - 8) % 3 == 0:
                        rt = (k - 8) // 3
                        bt = bp.tile([P, HW_D], F16, name="bt", tag="bt")
                        eng = nc.sync if rt % 2 == 0 else nc.scalar
                        eng.dma_start(
                            out=bt[:],
                            in_=bal.ap()[rt, :, h * HW_D : (h + 1) * HW_D],
                        )
                        for mb in range(MB_D):
                            for ns in range(NS_D):
                                nc.tensor.matmul(
                                    ps[mb * NS_D + ns][:],
                                    lhsT=h_sb[:, rt, mb * P : (mb + 1) * P],
                                    rhs=bt[:, ns * 512 : (ns + 1) * 512],
                                    start=False,
                                    stop=False,
                                )
                for mb in range(MB_D):
                    ot = op_.tile([P, HW_D], F16, name="ot", tag="ot")
                    for ns in range(NS_D):
                        if ns % 2 == 0:
                            nc.vector.tensor_copy(
                                out=ot[:, ns * 512 : (ns + 1) * 512],
                                in_=ps[mb * NS_D + ns][:],
                            )
                        else:
                            nc.scalar.copy(
                                out=ot[:, ns * 512 : (ns + 1) * 512],
                                in_=ps[mb * NS_D + ns][:],
                            )
                        if ns == 1:
                            nc.sync.dma_start(
                                out=y.ap()[
                                    mb * P : (mb + 1) * P,
                                    h * HW_D : h * HW_D + 1024,
                                ],
                                in_=ot[:, :1024],
                            )
                    nc.scalar.dma_start(
                        out=y.ap()[
                            mb * P : (mb + 1) * P,
                            h * HW_D + 1024 : (h + 1) * HW_D,
                        ],
                        in_=ot[:, 1024:],
                    )
            psp.release()

    nc.compile()
    return nc


def _kernel_dense(x, wids, lora_A, lora_B, M):
    x = np.asarray(x).reshape(B, IN).astype(np.float16, copy=False)
    wids = np.asarray(wids).reshape(B)
    lora_A = np.asarray(lora_A).astype(np.float16, copy=False)
    lora_B = np.asarray(lora_B).astype(np.float16, copy=False)
    M = np.asarray(M).astype(np.float16, copy=False)

    aal_np = np.ascontiguousarray(
        lora_A.transpose(1, 0, 2).reshape(IN, NR).reshape(KT_D, P, NR)
    )
    bal_np = np.ascontiguousarray(
        (lora_B * np.float16(2.0)).reshape(NR, OUT).reshape(RT_D, P, OUT)
    )
    mw_np = np.ascontiguousarray(M.reshape(KT_D, P, OUT))
    ra_np = (
        (np.arange(RT_D)[None, :] * P + np.arange(P)[:, None]) // R
    ).astype(np.float32)

    in_maps = []
    for c in range(N_CORES):
        xs = x[c * BC : (c + 1) * BC]
        xt_np = np.ascontiguousarray(xs.T.reshape(KT_D, P, BC).transpose(1, 0, 2))
        wd_np = np.ascontiguousarray(
            np.broadcast_to(
                wids[c * BC : (c + 1) * BC].astype(np.float16)[None, :], (P, BC)
            )
        )
        in_maps.append(
            {
                "xt": xt_np,
                "wd": wd_np,
                "ra": ra_np,
                "aal": aal_np,
                "bal": bal_np,
                "mw": mw_np,
            }
        )
    nc = build_nc_dense()
    res = run_bass_kernel_spmd(nc, in_maps, core_ids=list(range(N_CORES)))
    y = np.concatenate([res.results[c]["y"] for c in range(N_CORES)], axis=0)
    return y.reshape(B, 1, OUT)
